# revision 1
# baseline (speedup 1.0000x reference)
"""ETNN messager layer on 8 Trainium2 NeuronCores.

Edge-parallel, receiver-sharded: host sorts edges by receiver; core k owns
receivers [k*12500,(k+1)*12500) and scatter-adds into its private slice.
Gathers/scatter use indirect_dma_start ([P,1] per-partition offsets, int32).
BN folded into W1 on host. Messages: silu(state @ W1f + b1f),
gate = sigmoid(msg @ W2 + b2). Receivers within a chunk are made distinct by
column-major spreading so CCE-add scatters never collide inside one
instruction; pads go to a dump row.
"""

import numpy as np

import concourse.tile as tile
from concourse import bacc, bass, mybir
from concourse.bass_utils import run_bass_kernel_spmd
from concourse.masks import make_identity

N = 100000
E = 500000
H = 128
INV = 16
NCORES = 8
NLOC = N // NCORES          # 12500 receivers per core
CHUNK = 2048
NCHUNK = 36
SLOTS = NCHUNK * CHUNK      # 73728 slots/core
ST = CHUNK // 128           # 16 subtiles per chunk
BN_EPS = 1e-5

_prog_cache = {}


def _build(b2val: float):
    key = round(b2val, 9)
    if key in _prog_cache:
        return _prog_cache[key]
    nc = bacc.Bacc("TRN2", target_bir_lowering=False, debug=False)
    dt = mybir.dt
    xs = nc.dram_tensor("xs", [N, H], dt.float32, kind="ExternalInput")
    xr = nc.dram_tensor("xr", [NLOC + 1, H], dt.float32, kind="ExternalInput")
    sidx = nc.dram_tensor("sidx", [128, SLOTS // 128], dt.int32, kind="ExternalInput")
    ridx = nc.dram_tensor("ridx", [128, SLOTS // 128], dt.int32, kind="ExternalInput")
    eat = nc.dram_tensor("eat", [INV + 1, SLOTS], dt.float32, kind="ExternalInput")
    wa = nc.dram_tensor("wa", [H, H], dt.float32, kind="ExternalInput")
    wb = nc.dram_tensor("wb", [H, H], dt.float32, kind="ExternalInput")
    wc = nc.dram_tensor("wc", [INV + 1, H], dt.float32, kind="ExternalInput")
    w2b = nc.dram_tensor("w2b", [128, H], dt.float32, kind="ExternalInput")
    out = nc.dram_tensor("out", [NLOC + 1, H], dt.float32, kind="ExternalOutput")

    with tile.TileContext(nc) as tc:
        with tc.tile_pool(name="const", bufs=1) as cp, \
             tc.tile_pool(name="gath", bufs=4) as gp, \
             tc.tile_pool(name="trans", bufs=4) as tp, \
             tc.tile_pool(name="ea", bufs=3) as ep, \
             tc.tile_pool(name="msg", bufs=2) as mp, \
             tc.tile_pool(name="small", bufs=4) as sp, \
             tc.tile_pool(name="psum", bufs=2, space="PSUM") as pp:
            wa_sb = cp.tile([H, H], dt.float32)
            wb_sb = cp.tile([H, H], dt.float32)
            wc_sb = cp.tile([INV + 1, H], dt.float32)
            w2_sb = cp.tile([128, H], dt.float32)
            si_sb = cp.tile([128, SLOTS // 128], dt.int32)
            ri_sb = cp.tile([128, SLOTS // 128], dt.int32)
            ident = cp.tile([128, 128], dt.float32)
            make_identity(nc, ident[:])
            nc.sync.dma_start(out=wa_sb[:], in_=wa[:, :])
            nc.sync.dma_start(out=wb_sb[:], in_=wb[:, :])
            nc.sync.dma_start(out=wc_sb[:], in_=wc[:, :])
            nc.sync.dma_start(out=w2_sb[:], in_=w2b[:, :])
            nc.sync.dma_start(out=si_sb[:], in_=sidx[:, :])
            nc.sync.dma_start(out=ri_sb[:], in_=ridx[:, :])

            for cl in range(NCHUNK):
                ea_sb = ep.tile([INV + 1, CHUNK], dt.float32, tag="ea")
                nc.sync.dma_start(
                    out=ea_sb[:], in_=eat[:, cl * CHUNK : (cl + 1) * CHUNK]
                )
                msg = mp.tile([128, ST, H], dt.float32, tag="m")
                tt = mp.tile([128, ST, H], dt.float32, tag="t")
                ff = mp.tile([128, ST, H], dt.float32, tag="f")
                red = sp.tile([128, ST], dt.float32, tag="red")
                gate = sp.tile([128, ST], dt.float32, tag="gate")
                for j in range(ST):
                    q0 = cl * ST + j  # subtile column in idx tensors
                    js = slice(j * 128, (j + 1) * 128)
                    gs = gp.tile([128, H], dt.float32, tag="gs")
                    gr = gp.tile([128, H], dt.float32, tag="gr")
                    nc.gpsimd.indirect_dma_start(
                        out=gs[:], out_offset=None, in_=xs[:, :],
                        in_offset=bass.IndirectOffsetOnAxis(
                            ap=si_sb[:, q0 : q0 + 1], axis=0),
                    )
                    nc.gpsimd.indirect_dma_start(
                        out=gr[:], out_offset=None, in_=xr[:, :],
                        in_offset=bass.IndirectOffsetOnAxis(
                            ap=ri_sb[:, q0 : q0 + 1], axis=0),
                    )
                    tps = pp.tile([128, H], dt.float32, tag="tps")
                    tpr = pp.tile([128, H], dt.float32, tag="tpr")
                    nc.tensor.transpose(out=tps[:], in_=gs[:], identity=ident[:])
                    nc.tensor.transpose(out=tpr[:], in_=gr[:], identity=ident[:])
                    tss = tp.tile([128, H], dt.float32, tag="tss")
                    trs = tp.tile([128, H], dt.float32, tag="trs")
                    nc.vector.tensor_copy(out=tss[:], in_=tps[:])
                    nc.vector.tensor_copy(out=trs[:], in_=tpr[:])
                    pm = pp.tile([128, H], dt.float32, tag="pm")
                    nc.tensor.matmul(out=pm[:], lhsT=tss[:], rhs=wa_sb[:],
                                     start=True, stop=False)
                    nc.tensor.matmul(out=pm[:], lhsT=trs[:], rhs=wb_sb[:],
                                     start=False, stop=False)
                    nc.tensor.matmul(out=pm[:], lhsT=ea_sb[:, js], rhs=wc_sb[:],
                                     start=False, stop=True)
                    sg = sp.tile([128, H], dt.float32, tag="sg")
                    nc.scalar.activation(
                        out=sg[:], in_=pm[:],
                        func=mybir.ActivationFunctionType.Sigmoid)
                    nc.vector.tensor_tensor(
                        out=msg[:, j, :], in0=pm[:], in1=sg[:],
                        op=mybir.AluOpType.mult)
                    nc.vector.tensor_tensor(
                        out=tt[:, j, :], in0=msg[:, j, :], in1=w2_sb[:],
                        op=mybir.AluOpType.mult)
                nc.vector.tensor_reduce(
                    out=red[:], in_=tt[:, :, :],
                    axis=mybir.AxisListType.X, op=mybir.AluOpType.add)
                nc.scalar.activation(
                    out=gate[:], in_=red[:],
                    func=mybir.ActivationFunctionType.Sigmoid, bias=b2val)
                for j in range(ST):
                    nc.vector.tensor_tensor(
                        out=ff[:, j, :], in0=msg[:, j, :],
                        in1=gate[:, j : j + 1].to_broadcast([128, H]),
                        op=mybir.AluOpType.mult)
                for j in range(ST):
                    q0 = cl * ST + j
                    nc.gpsimd.indirect_dma_start(
                        out=out[:, :],
                        out_offset=bass.IndirectOffsetOnAxis(
                            ap=ri_sb[:, q0 : q0 + 1], axis=0),
                        in_=ff[:, j, :], in_offset=None,
                        compute_op=mybir.AluOpType.add,
                    )
    nc.compile()
    _prog_cache[key] = nc
    return nc


def _host_prep(x_send, x_rec, index, edge_attr, bn_gamma, bn_beta, bn_mean,
               bn_var, W1, b1, W2, b2):
    s = np.asarray(index[0], dtype=np.int64)
    r = np.asarray(index[1], dtype=np.int64)
    ea = np.asarray(edge_attr, dtype=np.float32)

    scale = np.asarray(bn_gamma) / np.sqrt(np.asarray(bn_var) + BN_EPS)
    shift = np.asarray(bn_beta) - np.asarray(bn_mean) * scale
    W1f = (np.asarray(W1) * scale[:, None]).astype(np.float32)
    b1f = (np.asarray(b1) + shift @ np.asarray(W1)).astype(np.float32)

    xs_f = np.asarray(x_send, dtype=np.float32)
    wa = W1f[:H]
    wb = W1f[H : 2 * H]
    wc = np.concatenate([W1f[2 * H :], b1f[None, :]], axis=0)
    w2b = np.broadcast_to(np.asarray(W2, dtype=np.float32).reshape(1, H),
                          (128, H)).copy()
    b2val = float(np.asarray(b2).reshape(-1)[0])

    in_maps = []
    for k in range(NCORES):
        m = (r // NLOC) == k
        sk = s[m]
        rk = (r[m] - k * NLOC).astype(np.int64)
        eak = ea[m]
        n = sk.shape[0]
        assert n <= SLOTS, f"shard overflow {n}"
        xr_loc = np.zeros((NLOC + 1, H), dtype=np.float32)
        xr_loc[:NLOC] = np.asarray(x_rec[k * NLOC : (k + 1) * NLOC],
                                   dtype=np.float32)
        sidx = np.zeros((128, SLOTS // 128), dtype=np.int32)
        ridx = np.full((128, SLOTS // 128), NLOC, dtype=np.int32)
        eat = np.zeros((INV + 1, SLOTS), dtype=np.float32)
        eat[INV, :] = 1.0
        # sort by receiver, spread column-major over chunks so receivers are
        # distinct within each chunk (and each 128-subtile)
        o = np.argsort(rk, kind="stable")
        sk, rk, eak = sk[o], rk[o], eak[o]
        i = np.arange(n)
        c = i % NCHUNK
        q = i // NCHUNK          # slot within chunk, < 2048
        col = c * ST + q // 128  # subtile column
        row = q % 128            # partition
        sidx[row, col] = sk.astype(np.int32)
        ridx[row, col] = rk.astype(np.int32)
        eat[:INV, c * CHUNK + q] = eak.T
        in_maps.append({
            "xs": xs_f, "xr": xr_loc, "sidx": sidx, "ridx": ridx,
            "eat": eat, "wa": wa, "wb": wb, "wc": wc, "w2b": w2b,
        })
    return in_maps, b2val


def kernel(**inputs) -> np.ndarray:
    in_maps, b2val = _host_prep(**inputs)
    nc = _build(b2val)
    res = run_bass_kernel_spmd(nc, in_maps, core_ids=list(range(NCORES)))
    return np.concatenate(
        [res.results[k]["out"][:NLOC] for k in range(NCORES)], axis=0
    ).astype(np.float32)



# revision 10
# speedup vs baseline: 11319.5888x; 11319.5888x over previous
"""ETNN messager layer on 8 Trainium2 NeuronCores — v3 (all-SBUF random access).

Receiver-sharded: core k owns receivers [k*12500,(k+1)*12500). All random
(per-edge) access runs SBUF<->SBUF, where small packets do not pay HBM
latency; HBM sees only contiguous streams:

- x_send is split into 4 sender-range buckets of 25088 rows; each bucket's
  table (6.4MB bf16) is DMA-streamed into SBUF, and edges (grouped by
  bucket on host) gather rows via dma_gather (SBUF-source, int16 indices,
  transpose=True) which lands x^T tiles directly usable as matmul lhsT.
- P_rec = x_rec_loc @ Wb is computed on device and kept SBUF-resident;
  per-edge rows come from the same SBUF dma_gather, accumulated into PSUM
  via an identity matmul (layout fix [H,e]->[e,H]).
- The segment-sum runs via dma_scatter_add into SBUF fp16 accumulators
  (parity-split even/odd rank tiles), flushed contiguously at the end.

Host sorts each bucket by receiver and round-robins edges over the
bucket's 2048-slot scatter windows so receivers are distinct within every
scatter instruction; receivers whose per-bucket degree exceeds the window
count (a handful of edges in pathological cases) are computed on host and
added to the output.

msg = sigmoid(z)*z with z = xs[s]@Wa + ea@Wc(+b1) + P_rec[r] (BN folded
into W1 on host); gate = sigmoid(msg.W2+b2); out[r] += msg*gate.
"""

import numpy as np
from ml_dtypes import bfloat16

import concourse.tile as tile
from concourse import bacc, bass, mybir
from concourse.bass_utils import run_bass_kernel_spmd
from concourse.masks import make_identity

N = 100000
E = 500000
H = 128
INV = 16
NCORES = 8
NLOC = N // NCORES           # 12500 receivers per core
NRANK = 98                   # receiver ranks of 128 rows (12544)
DUMP = NRANK * 128           # scatter/gather dump index (rank 98)
NBKT = 4
BROWS = 25088                # bucket rows (196 ranks of 128)
BRANK = 196
GWIN = 4096                  # gather granularity (2 scatter windows)
SWIN = 2048                  # scatter window
BN_EPS = 1e-5

_prog_cache = {}


def _build(b2val: float, nws: tuple):
    """nws: per-bucket window counts (each even); slots = sum(nws)*2048."""
    key = (round(b2val, 9), nws)
    if key in _prog_cache:
        return _prog_cache[key]
    nwtot = sum(nws)
    slots = nwtot * SWIN
    nsub = SWIN // 128       # 16 subtiles per window

    nc = bacc.Bacc("TRN2", target_bir_lowering=False, debug=False)
    dt = mybir.dt
    xs4 = nc.dram_tensor("xs4", [NBKT, 128, BROWS], dt.bfloat16,
                         kind="ExternalInput")
    xrt = nc.dram_tensor("xrt", [128, NRANK * 128], dt.bfloat16,
                         kind="ExternalInput")
    eat = nc.dram_tensor("eat", [INV + 1, slots], dt.bfloat16,
                         kind="ExternalInput")
    sg16 = nc.dram_tensor("sg16", [128, slots // 16], dt.int16,
                          kind="ExternalInput")
    rg16 = nc.dram_tensor("rg16", [128, slots // 16], dt.int16,
                          kind="ExternalInput")
    wa = nc.dram_tensor("wa", [H, H], dt.bfloat16, kind="ExternalInput")
    wb = nc.dram_tensor("wb", [H, H], dt.bfloat16, kind="ExternalInput")
    wc = nc.dram_tensor("wc", [INV + 1, H], dt.bfloat16, kind="ExternalInput")
    w2w = nc.dram_tensor("w2w", [128, nsub, H], dt.bfloat16,
                         kind="ExternalInput")
    oed = nc.dram_tensor("oed", [128, 50 * H], dt.float16,
                         kind="ExternalOutput")
    ood = nc.dram_tensor("ood", [128, 50 * H], dt.float16,
                         kind="ExternalOutput")

    with tile.TileContext(nc) as tc:
        with tc.tile_pool(name="const", bufs=1) as cp, \
             tc.tile_pool(name="xsb", bufs=1) as xsp, \
             tc.tile_pool(name="xr", bufs=3) as xrp, \
             tc.tile_pool(name="idx", bufs=3) as ixp, \
             tc.tile_pool(name="gath", bufs=2) as gp, \
             tc.tile_pool(name="ea", bufs=2) as eap, \
             tc.tile_pool(name="sgp", bufs=4) as sgp, \
             tc.tile_pool(name="msg", bufs=2) as mp, \
             tc.tile_pool(name="ff", bufs=2) as fp_, \
             tc.tile_pool(name="small", bufs=3) as sp, \
             tc.tile_pool(name="pmp", bufs=2, space="PSUM") as pmp, \
             tc.tile_pool(name="prp_ps", bufs=2, space="PSUM") as prpp:
            ident = cp.tile([128, 128], dt.bfloat16)
            make_identity(nc, ident[:])
            wa_sb = cp.tile([H, H], dt.bfloat16)
            wb_sb = cp.tile([H, H], dt.bfloat16)
            wc_sb = cp.tile([INV + 1, H], dt.bfloat16)
            w2w_sb = cp.tile([128, nsub, H], dt.bfloat16)
            b2t = cp.tile([128, 1], dt.float32)
            nc.gpsimd.memset(b2t[:], b2val)
            nc.sync.dma_start(out=wa_sb[:], in_=wa[:, :])
            nc.sync.dma_start(out=wb_sb[:], in_=wb[:, :])
            nc.sync.dma_start(out=wc_sb[:], in_=wc[:, :])
            nc.sync.dma_start(out=w2w_sb[:], in_=w2w[:, :, :])

            # output accumulators (parity-split by receiver rank)
            oe = cp.tile([128, 50, H], dt.float16)
            oo = cp.tile([128, 50, H], dt.float16)
            nc.gpsimd.memset(oe[:], 0.0)
            nc.gpsimd.memset(oo[:], 0.0)

            # phase 1: P_rec = x_rec_loc @ Wb -> SBUF resident [128, 99, 128]
            prec = cp.tile([128, NRANK + 1, H], dt.bfloat16)
            nc.vector.memset(prec[:, NRANK, :], 0.0)
            for t in range(NRANK):
                xrt_sb = xrp.tile([128, 128], dt.bfloat16, tag="xrt")
                nc.sync.dma_start(
                    out=xrt_sb[:], in_=xrt[:, t * 128 : (t + 1) * 128]
                )
                ppr = prpp.tile([128, 128], dt.float32, tag="ppr")
                nc.tensor.matmul(out=ppr[:], lhsT=xrt_sb[:], rhs=wb_sb[:],
                                 start=True, stop=True)
                nc.vector.tensor_copy(out=prec[:, t, :], in_=ppr[:])

            # phase 2: bucket-major edge pipeline
            W = 0  # global scatter-window index
            for b in range(NBKT):
                xs_sb = xsp.tile([128, BROWS], dt.bfloat16, tag="xs")
                nc.sync.dma_start(out=xs_sb[:], in_=xs4[b, :, :])
                for wg in range(nws[b] // 2):   # gather groups of 4096
                    gbase = W * SWIN            # slot base of this group
                    icol = slice(gbase // 16, (gbase + GWIN) // 16)
                    sg_sb = ixp.tile([128, GWIN // 16], dt.int16, tag="sg")
                    rg_sb = ixp.tile([128, GWIN // 16], dt.int16, tag="rg")
                    nc.sync.dma_start(out=sg_sb[:], in_=sg16[:, icol])
                    nc.sync.dma_start(out=rg_sb[:], in_=rg16[:, icol])
                    gsT = gp.tile([128, 1, GWIN], dt.bfloat16, tag="gsT")
                    nc.gpsimd.dma_gather(
                        out_ap=gsT[:], in_ap=xs_sb[:], idxs_ap=sg_sb[:],
                        num_idxs=GWIN, num_idxs_reg=GWIN, elem_size=H,
                        transpose=True, sbuf_tokens_per_rank=128,
                        sbuf_free_dim_per_rank=256, single_packet=False,
                    )
                    grT = gp.tile([128, 1, GWIN], dt.bfloat16, tag="grT")
                    nc.gpsimd.dma_gather(
                        out_ap=grT[:], in_ap=prec[:], idxs_ap=rg_sb[:],
                        num_idxs=GWIN, num_idxs_reg=GWIN, elem_size=H,
                        transpose=True, sbuf_tokens_per_rank=128,
                        sbuf_free_dim_per_rank=256, single_packet=False,
                    )
                    ea_sb = eap.tile([INV + 1, GWIN], dt.bfloat16, tag="ea")
                    nc.sync.dma_start(
                        out=ea_sb[:], in_=eat[:, gbase : gbase + GWIN]
                    )
                    for w in range(2):          # scatter windows in group
                        msg = mp.tile([128, nsub, H], dt.bfloat16, tag="msg")
                        for g in range(nsub // 4):
                            pm = pmp.tile([128, 4, 128], dt.float32, tag="pm")
                            for jj in range(4):
                                j = w * nsub + 4 * g + jj
                                js = slice(j * 128, (j + 1) * 128)
                                nc.tensor.matmul(
                                    out=pm[:, jj, :], lhsT=gsT[:, 0, js],
                                    rhs=wa_sb[:], start=True, stop=False)
                                nc.tensor.matmul(
                                    out=pm[:, jj, :], lhsT=ea_sb[:, js],
                                    rhs=wc_sb[:], start=False, stop=False)
                                nc.tensor.matmul(
                                    out=pm[:, jj, :], lhsT=grT[:, 0, js],
                                    rhs=ident[:], start=False, stop=True)
                            g4 = slice(4 * g, 4 * g + 4)
                            sg_t = sgp.tile([128, 4, 128], dt.bfloat16,
                                            tag="sg_t")
                            nc.scalar.activation(
                                out=sg_t[:], in_=pm[:, :, :],
                                func=mybir.ActivationFunctionType.Sigmoid)
                            nc.vector.tensor_tensor(
                                out=msg[:, g4, :], in0=pm[:, :, :],
                                in1=sg_t[:], op=mybir.AluOpType.mult)
                        tt = mp.tile([128, nsub, H], dt.bfloat16, tag="tt")
                        nc.vector.tensor_tensor(
                            out=tt[:], in0=msg[:], in1=w2w_sb[:],
                            op=mybir.AluOpType.mult)
                        red = sp.tile([128, nsub], dt.float32, tag="red")
                        nc.vector.tensor_reduce(
                            out=red[:], in_=tt[:],
                            axis=mybir.AxisListType.X, op=mybir.AluOpType.add)
                        gate = sp.tile([128, nsub], dt.bfloat16, tag="gate")
                        nc.scalar.activation(
                            out=gate[:], in_=red[:],
                            func=mybir.ActivationFunctionType.Sigmoid,
                            bias=b2t[:, 0:1])
                        ffl = fp_.tile([128, nsub, H], dt.float16, tag="ff")
                        nc.vector.tensor_tensor(
                            out=ffl[:], in0=msg[:],
                            in1=gate[:].to_broadcast([128, nsub, H]),
                            op=mybir.AluOpType.mult)
                        nc.gpsimd.dma_scatter_add(
                            out_ap=oe[:], in_ap=ffl[:],
                            idxs_ap=rg_sb[:, w * 128 : (w + 1) * 128],
                            num_idxs=SWIN, num_idxs_reg=SWIN, elem_size=H,
                            parity_reg=0, out_ap_other=oo[:],
                            sbuf_tokens_per_rank=128,
                        )
                        W += 1
            nc.sync.dma_start(out=oed[:, :], in_=oe[:])
            nc.sync.dma_start(out=ood[:, :], in_=oo[:])
    nc.compile()
    _prog_cache[key] = nc
    return nc


def _host_prep(x_send, x_rec, index, edge_attr, bn_gamma, bn_beta, bn_mean,
               bn_var, W1, b1, W2, b2):
    s = np.asarray(index[0], dtype=np.int64)
    r = np.asarray(index[1], dtype=np.int64)
    ea = np.asarray(edge_attr, dtype=np.float32)

    scale = np.asarray(bn_gamma) / np.sqrt(np.asarray(bn_var) + BN_EPS)
    shift = np.asarray(bn_beta) - np.asarray(bn_mean) * scale
    W1f = (np.asarray(W1) * scale[:, None]).astype(np.float32)
    b1f = (np.asarray(b1) + shift @ np.asarray(W1)).astype(np.float32)

    wa = W1f[:H].astype(bfloat16)
    wb = W1f[H : 2 * H].astype(bfloat16)
    wc = np.concatenate([W1f[2 * H :], b1f[None, :]], axis=0).astype(bfloat16)
    nsub = SWIN // 128
    w2w = np.broadcast_to(
        np.asarray(W2, dtype=np.float32).reshape(1, 1, H), (128, nsub, H)
    ).astype(bfloat16).copy()
    b2val = float(np.asarray(b2).reshape(-1)[0])

    xs_f = np.asarray(x_send, dtype=np.float32)
    xr_f = np.asarray(x_rec, dtype=np.float32)
    xs4 = np.zeros((NBKT, 128, BROWS), dtype=bfloat16)
    for bb in range(NBKT):
        rows = xs_f[bb * 25000 : (bb + 1) * 25000].astype(bfloat16)  # [25000,128]
        n = rows.shape[0]
        i = np.arange(n)
        xs4[bb].reshape(128, BRANK, 128)[i % 128, i // 128, :] = rows

    core_of = (r // NLOC).astype(np.int64)
    bkt_of = (s // 25000).astype(np.int64)

    # common per-bucket window counts across cores (SPMD single program)
    nws = []
    for bb in range(NBKT):
        nb_max = max(
            int(((core_of == k) & (bkt_of == bb)).sum()) for k in range(NCORES)
        )
        nw = -(-nb_max // SWIN)
        nws.append(nw + (nw & 1))  # round up to even
    nws = tuple(nws)
    slots = sum(nws) * SWIN

    in_maps = []
    absorbed = []  # (global_receiver_row, contribution) host-computed edges
    for k in range(NCORES):
        sg16 = np.zeros((16, slots // 16), dtype=np.int16)
        rg16 = np.full((16, slots // 16), DUMP, dtype=np.int16)
        eat = np.zeros((INV + 1, slots), dtype=np.float32)
        wbase = 0
        for bb in range(NBKT):
            m = (core_of == k) & (bkt_of == bb)
            sk = s[m] - bb * 25000
            rk = (r[m] - k * NLOC).astype(np.int64)
            eak = ea[m]
            nw = nws[bb]
            o = np.argsort(rk, kind="stable")
            sk, rk, eak = sk[o], rk[o], eak[o]
            # host-absorb edges whose receiver degree exceeds nw
            i = np.arange(sk.shape[0])
            first = np.searchsorted(rk, rk, side="left")
            occ = i - first  # occurrence number of this receiver
            keep = occ < nw
            if not keep.all():
                gidx = np.where(m)[0][o][~keep]
                absorbed.extend(gidx.tolist())
            sk, rk, eak = sk[keep], rk[keep], eak[keep]
            n = sk.shape[0]
            i = np.arange(n)
            w = i % nw
            q = i // nw
            slot = (wbase + w) * SWIN + q
            sg16[slot % 16, slot // 16] = sk.astype(np.int16)
            rg16[slot % 16, slot // 16] = rk.astype(np.int16)
            eat[:INV, slot] = eak.T
            eat[INV, slot] = 1.0
            wbase += nw
        xrt = np.zeros((128, NRANK * 128), dtype=np.float32)
        xrt[:, :NLOC] = xr_f[k * NLOC : (k + 1) * NLOC].T
        in_maps.append({
            "xs4": xs4, "xrt": xrt.astype(bfloat16),
            "eat": eat.astype(bfloat16),
            "sg16": np.tile(sg16, (8, 1)), "rg16": np.tile(rg16, (8, 1)),
            "wa": wa, "wb": wb, "wc": wc, "w2w": w2w,
        })

    # host-computed contributions for absorbed edges (rare)
    corr = None
    if absorbed:
        ai = np.asarray(absorbed, dtype=np.int64)
        st = np.concatenate(
            [xs_f[s[ai]], xr_f[r[ai]], ea[ai]], axis=1)
        z = st @ W1f + b1f
        msg = z / (1.0 + np.exp(-z))
        gate = 1.0 / (1.0 + np.exp(-(msg @ np.asarray(W2).reshape(H, 1)
                                     + b2val)))
        corr = (ai, r[ai], msg * gate)
    return in_maps, b2val, nws, corr


def _unpack(oed, ood):
    """[128, 50, 128] even/odd rank tiles -> [12500, 128] rows."""
    out = np.empty((NLOC, H), dtype=np.float32)
    oe = np.asarray(oed, dtype=np.float32).reshape(128, 50, H)
    oo = np.asarray(ood, dtype=np.float32).reshape(128, 50, H)
    r = np.arange(NRANK * 128)
    tok, rank = r % 128, r // 128
    vals = np.where((rank % 2 == 0)[:, None],
                    oe[tok, rank // 2, :], oo[tok, rank // 2, :])
    return vals[:NLOC]


def kernel(**inputs) -> np.ndarray:
    in_maps, b2val, nws, corr = _host_prep(**inputs)
    nc = _build(b2val, nws)
    res = run_bass_kernel_spmd(nc, in_maps, core_ids=list(range(NCORES)))
    out = np.concatenate(
        [_unpack(res.results[k]["oed"], res.results[k]["ood"])
         for k in range(NCORES)], axis=0
    )
    if corr is not None:
        _, rows, contrib = corr
        np.add.at(out, rows, contrib.astype(np.float32))
    return out
